# revision 17
# baseline (speedup 1.0000x reference)
"""Distance-discriminator kernel for 8 Trainium2 cores.

Math (reference): for x [N, D],
    S[d] = sum_j x[j,d];  Q[d] = sum_j x[j,d]^2
    sq[i,d] = Q[d] - 2 x[i,d] S[d] + N x[i,d]^2        (= sum_j (x[j,d]-x[i,d])^2)
    out = log(sqrt(sq) + eps) @ W.T + b

Device formulation: complete the square,
    sq = (sqrt(N) x - S/sqrt(N))^2 + C,   C = Q - S^2/N
so one ACT Square pass (per-partition bias, accum_out gives N*C for free) and
one ACT Ln pass (per-partition bias C) produce logd2 = ln(sq) = 2 log(dist).
The eps and the 0.5 factor fold into the GEMM weights (W/2); eps is
negligible because dist ~ sqrt(2N) >> eps.

Sharding: columns d are split across the 8 cores (512 each), so S, Q are
fully local; each core computes a [64, 4096] partial of out.T and a
ReduceScatter sums partials, leaving rank c with out.T rows 8c:8c+8.
Inputs are transposed on the host so d sits on SBUF partitions: reductions
are free-axis, the GEMM needs no on-device transpose, and all DMA is
contiguous.
"""

import numpy as np

import concourse.bacc as bacc
import concourse.bass as bass
import concourse.tile as tile
from concourse import mybir
from concourse.tile import add_dep_helper
from concourse.bass_utils import run_bass_kernel_spmd

N = 4096          # rows
D = 4096          # feature columns
OUT = 64
NCORES = 8
DC = D // NCORES  # 512 columns per core
KCH = DC // 128   # 4 partition-chunks per core
NBLK = N // 512   # 8 moving-dim blocks per GEMM bank
SQRT_N = float(np.sqrt(N))
C0 = 8.9              # ln(sq) centering constant; absorbed via host bias
EMC0 = float(np.exp(-C0))

F32 = mybir.dt.float32
_cache: dict = {}


def _build():
    nc = bacc.Bacc(
        "TRN2",
        target_bir_lowering=False,
        debug=False,
        num_devices=NCORES,
    )
    xT = nc.dram_tensor("xT", [DC, N], F32, kind="ExternalInput").ap()
    wT = nc.dram_tensor("wT", [DC, OUT], F32, kind="ExternalInput").ap()
    bb = nc.dram_tensor("bb", [OUT, 1], F32, kind="ExternalInput").ap()
    out = nc.dram_tensor("out", [OUT, N], F32, kind="ExternalOutput").ap()

    F32R = mybir.dt.float32r
    with tile.TileContext(nc) as tc:
        with (
            tc.tile_pool(name="wp", bufs=1) as wp,
            tc.tile_pool(name="xp", bufs=KCH) as xp,
            tc.tile_pool(name="st", bufs=KCH) as st,
            tc.tile_pool(name="up", bufs=KCH) as up,
            tc.tile_pool(name="lp", bufs=3) as lp,
            tc.tile_pool(name="pp", bufs=NBLK, space="PSUM") as pp,
        ):
            # pre-load both ACT table sets (Square, Ln) while ACT is idle;
            # ordering deps below force these to schedule first
            dumm = wp.tile([128, 1], F32, name="dumm", tag="dumm")
            nc.vector.memset(dumm[:], 1.0)
            dumm2 = wp.tile([128, 1], F32, name="dumm2", tag="dumm2")
            pre_sq = nc.scalar.activation(
                dumm2[:], dumm[:], mybir.ActivationFunctionType.Square
            )
            pre_ln = nc.scalar.activation(
                dumm2[:], dumm[:], mybir.ActivationFunctionType.Ln,
                bias=dumm[:], scale=1.0,
            )

            xs = []
            di = 0
            for k in range(KCH):
                x_k = xp.tile([128, N], F32, name=f"x_{k}", tag="x")
                npieces = 8 if k == 0 else 4
                w_piece = N // npieces
                for s in range(npieces):
                    # spread issue across engine queues: per-queue data BW is
                    # ~220 GB/s; scalar participates only before its ACT work
                    if k == 0:
                        eng = [nc.sync, nc.scalar, nc.gpsimd][di % 3]
                    else:
                        eng = [nc.sync, nc.gpsimd, nc.sync, nc.scalar][di % 4] if di < 12 else [nc.sync, nc.gpsimd][di % 2]
                    di += 1
                    eng.dma_start(
                        x_k[:, s * w_piece : (s + 1) * w_piece],
                        xT[k * 128 : (k + 1) * 128, s * w_piece : (s + 1) * w_piece],
                    )
                xs.append(x_k)

            w_all = wp.tile([128, KCH * OUT], F32, name="w_all", tag="w_all")
            for k in range(KCH):
                nc.sync.dma_start(
                    w_all[:, k * OUT : (k + 1) * OUT], wT[k * 128 : (k + 1) * 128, :]
                )
            bias_b = wp.tile([OUT, 1], F32, name="bias_b", tag="bias_b")
            nc.sync.dma_start(bias_b[:], bb)
            w_r = wp.tile([128, KCH * OUT], F32R, name="w_r", tag="w_r")
            nc.vector.tensor_copy(w_r[:], w_all[:])

            us, Cs = [], []
            for k in range(KCH):
                x_k = xs[k]
                # bn_stats per 512-wide segment -> mean/var per partition
                stats_k = st.tile([128, 8, 6], F32, name=f"stats_{k}", tag="stats")
                for s in range(8):
                    nc.vector.bn_stats(
                        stats_k[:, s, :], x_k[:, s * 512 : (s + 1) * 512]
                    )
                mv_k = st.tile([128, 2], F32, name=f"mv_{k}", tag="mv")
                nc.vector.bn_aggr(mv_k[:], stats_k[:])
                # bias_A = -S/sqrt(N) = -sqrt(N)*mean ;  C = Q - S^2/N = N*var
                bA_k = st.tile([128, 1], F32, name=f"bA_{k}", tag="bA")
                nc.vector.tensor_scalar_mul(bA_k[:], mv_k[:, 0:1], -SQRT_N)
                C_k = st.tile([128, 1], F32, name=f"C_{k}", tag="C")
                nc.vector.tensor_scalar_mul(C_k[:], mv_k[:, 1:2], float(N) * EMC0)
                u_k = up.tile([128, N], F32, name=f"u_{k}", tag="u")
                nsq = 2 if k == KCH - 1 else 1
                wsq = N // nsq
                for hh in range(nsq):
                    sq_i = nc.scalar.activation(
                        u_k[:, hh * wsq : (hh + 1) * wsq],
                        x_k[:, hh * wsq : (hh + 1) * wsq],
                        mybir.ActivationFunctionType.Square,
                        bias=bA_k[:],
                        scale=SQRT_N,
                    )
                    if k == 0 and hh == 0:
                        add_dep_helper(
                            sq_i.ins, pre_sq.ins, sync=False,
                            reason="table preload first",
                        )
                        add_dep_helper(
                            sq_i.ins, pre_ln.ins, sync=False,
                            reason="table preload first",
                        )
                us.append(u_k)
                Cs.append(C_k)

            psums = [
                pp.tile([OUT, 512], F32, name=f"ps_{j}", tag="ps")
                for j in range(NBLK)
            ]
            out_sb = wp.tile([OUT, N], F32, name="out_sb", tag="out_sb")
            HB = NBLK // 2  # n-blocks per half
            for h in range(2):
                for k in range(KCH):
                    l_k = lp.tile([128, N // 2], F32R, name=f"l_{h}_{k}", tag="l")
                    # finer Ln pieces on the last chunk shorten the end drain
                    nq = 2 if k == KCH - 1 else 1
                    wq = (N // 2) // nq
                    for q in range(nq):
                        nc.scalar.activation(
                            l_k[:, q * wq : (q + 1) * wq],
                            us[k][
                                :, h * (N // 2) + q * wq : h * (N // 2) + (q + 1) * wq
                            ],
                            mybir.ActivationFunctionType.Ln,
                            bias=Cs[k][:],
                            scale=EMC0,
                        )
                    for jj in range(HB):
                        j = h * HB + jj
                        nc.tensor.matmul(
                            psums[j][:],
                            lhsT=w_r[:, k * OUT : (k + 1) * OUT],
                            rhs=l_k[:, jj * 512 : (jj + 1) * 512],
                            start=(k == 0),
                            stop=(k == KCH - 1),
                        )
                for jj in range(HB):
                    j = h * HB + jj
                    if h == 0 or jj % 2 == 0:
                        nc.vector.tensor_scalar_add(
                            out_sb[:, j * 512 : (j + 1) * 512], psums[j][:], bias_b[:]
                        )
                    else:
                        nc.scalar.add(
                            out_sb[:, j * 512 : (j + 1) * 512], psums[j][:], bias_b[:]
                        )
                if h == 0:
                    nc.sync.dma_start(
                        out[:, h * (N // 2) : (h + 1) * (N // 2)],
                        out_sb[:, h * (N // 2) : (h + 1) * (N // 2)],
                    )
                else:
                    # per-bank output DMAs chase the evacuations at the tail
                    for jj in range(HB):
                        j = h * HB + jj
                        eng = [nc.sync, nc.gpsimd][jj % 2]
                        eng.dma_start(
                            out[:, j * 512 : (j + 1) * 512],
                            out_sb[:, j * 512 : (j + 1) * 512],
                        )

    nc.compile()
    return nc


def _prep_inputs(data, W, b):
    data = np.ascontiguousarray(np.asarray(data, dtype=np.float32))
    W = np.asarray(W, dtype=np.float32)
    b = np.asarray(b, dtype=np.float32)
    W2T = np.ascontiguousarray(W.T * 0.5)          # [D, OUT]
    in_maps = []
    for c in range(NCORES):
        xT_c = np.ascontiguousarray(data[:, c * DC : (c + 1) * DC].T)  # [DC, N]
        wT_c = np.ascontiguousarray(W2T[c * DC : (c + 1) * DC, :])     # [DC, OUT]
        # bias per core: b/8 plus the centering correction C0*sum_d w2[d,o]
        b8_c = (b / NCORES + C0 * wT_c.sum(axis=0)).astype(np.float32)
        in_maps.append({"xT": xT_c, "wT": wT_c, "bb": np.ascontiguousarray(b8_c.reshape(OUT, 1))})
    return in_maps


def _run(inputs, trace=False, **kwargs):
    if "nc" not in _cache:
        _cache["nc"] = _build()
    nc = _cache["nc"]
    in_maps = _prep_inputs(inputs["data"], inputs["W"], inputs["b"])
    res = run_bass_kernel_spmd(
        nc, in_maps, core_ids=list(range(NCORES)), trace=trace, **kwargs
    )
    outT = np.sum([res.results[c]["out"] for c in range(NCORES)], axis=0, dtype=np.float32)
    return np.ascontiguousarray(outT.T), res


def kernel(data, W, b):
    out, _ = _run({"data": data, "W": W, "b": b})
    return out


# revision 18
# speedup vs baseline: 1.1810x; 1.1810x over previous
"""Distance-discriminator kernel for 8 Trainium2 cores.

Math (reference): for x [N, D],
    S[d] = sum_j x[j,d];  Q[d] = sum_j x[j,d]^2
    sq[i,d] = Q[d] - 2 x[i,d] S[d] + N x[i,d]^2        (= sum_j (x[j,d]-x[i,d])^2)
    out = log(sqrt(sq) + eps) @ W.T + b

Device formulation: complete the square,
    sq = (sqrt(N) x - S/sqrt(N))^2 + C,   C = Q - S^2/N
so one ACT Square pass (per-partition bias, accum_out gives N*C for free) and
one ACT Ln pass (per-partition bias C) produce logd2 = ln(sq) = 2 log(dist).
The eps and the 0.5 factor fold into the GEMM weights (W/2); eps is
negligible because dist ~ sqrt(2N) >> eps.

Sharding: columns d are split across the 8 cores (512 each), so S, Q are
fully local; each core computes a [64, 4096] partial of out.T and a
ReduceScatter sums partials, leaving rank c with out.T rows 8c:8c+8.
Inputs are transposed on the host so d sits on SBUF partitions: reductions
are free-axis, the GEMM needs no on-device transpose, and all DMA is
contiguous.
"""

import numpy as np

import concourse.bacc as bacc
import concourse.bass as bass
import concourse.tile as tile
from concourse import mybir
from concourse.tile import add_dep_helper
from concourse.bass_utils import run_bass_kernel_spmd

N = 4096          # rows
D = 4096          # feature columns
OUT = 64
NCORES = 8
DC = D // NCORES  # 512 columns per core
KCH = DC // 128   # 4 partition-chunks per core
NBLK = N // 512   # 8 moving-dim blocks per GEMM bank
SQRT_N = float(np.sqrt(N))
C0 = 8.9              # ln(sq) centering constant; absorbed via host bias
EMC0 = float(np.exp(-C0))

F32 = mybir.dt.float32
_cache: dict = {}


def _build():
    nc = bacc.Bacc(
        "TRN2",
        target_bir_lowering=False,
        debug=False,
        num_devices=NCORES,
    )
    xT = nc.dram_tensor("xT", [DC, N], F32, kind="ExternalInput").ap()
    wT = nc.dram_tensor("wT", [DC, OUT], F32, kind="ExternalInput").ap()
    bb = nc.dram_tensor("bb", [OUT, 1], F32, kind="ExternalInput").ap()
    out = nc.dram_tensor("out", [OUT, N], F32, kind="ExternalOutput").ap()

    F32R = mybir.dt.float32r
    with tile.TileContext(nc) as tc:
        with (
            tc.tile_pool(name="wp", bufs=1) as wp,
            tc.tile_pool(name="xp", bufs=KCH) as xp,
            tc.tile_pool(name="st", bufs=KCH) as st,
            tc.tile_pool(name="up", bufs=KCH) as up,
            tc.tile_pool(name="lp", bufs=3) as lp,
            tc.tile_pool(name="pp", bufs=NBLK, space="PSUM") as pp,
        ):
            # pre-load both ACT table sets (Square, Ln) while ACT is idle;
            # ordering deps below force these to schedule first
            dumm = wp.tile([128, 1], F32, name="dumm", tag="dumm")
            nc.vector.memset(dumm[:], 1.0)
            dumm2 = wp.tile([128, 1], F32, name="dumm2", tag="dumm2")
            pre_sq = nc.scalar.activation(
                dumm2[:], dumm[:], mybir.ActivationFunctionType.Square
            )
            pre_ln = nc.scalar.activation(
                dumm2[:], dumm[:], mybir.ActivationFunctionType.Ln,
                bias=dumm[:], scale=1.0,
            )

            xs = []
            di = 0
            for k in range(KCH):
                x_k = xp.tile([128, N], F32, name=f"x_{k}", tag="x")
                npieces = 8 if k == 0 else 4
                w_piece = N // npieces
                for s in range(npieces):
                    # alternate issuing engine: per-queue data BW is ~220 GB/s;
                    # scalar participates only before its ACT work starts
                    eng = nc.scalar if (di % 2 == 1 and di < 12) else nc.sync
                    di += 1
                    eng.dma_start(
                        x_k[:, s * w_piece : (s + 1) * w_piece],
                        xT[k * 128 : (k + 1) * 128, s * w_piece : (s + 1) * w_piece],
                    )
                xs.append(x_k)

            w_all = wp.tile([128, KCH * OUT], F32, name="w_all", tag="w_all")
            for k in range(KCH):
                nc.sync.dma_start(
                    w_all[:, k * OUT : (k + 1) * OUT], wT[k * 128 : (k + 1) * 128, :]
                )
            bias_b = wp.tile([OUT, 1], F32, name="bias_b", tag="bias_b")
            nc.sync.dma_start(bias_b[:], bb)
            w_r = wp.tile([128, KCH * OUT], F32R, name="w_r", tag="w_r")
            nc.vector.tensor_copy(w_r[:], w_all[:])

            us, Cs = [], []
            for k in range(KCH):
                x_k = xs[k]
                # bn_stats per 512-wide segment -> mean/var per partition
                stats_k = st.tile([128, 8, 6], F32, name=f"stats_{k}", tag="stats")
                for s in range(8):
                    nc.vector.bn_stats(
                        stats_k[:, s, :], x_k[:, s * 512 : (s + 1) * 512]
                    )
                mv_k = st.tile([128, 2], F32, name=f"mv_{k}", tag="mv")
                nc.vector.bn_aggr(mv_k[:], stats_k[:])
                # bias_A = -S/sqrt(N) = -sqrt(N)*mean ;  C = Q - S^2/N = N*var
                bA_k = st.tile([128, 1], F32, name=f"bA_{k}", tag="bA")
                nc.vector.tensor_scalar_mul(bA_k[:], mv_k[:, 0:1], -SQRT_N)
                C_k = st.tile([128, 1], F32, name=f"C_{k}", tag="C")
                nc.vector.tensor_scalar_mul(C_k[:], mv_k[:, 1:2], float(N) * EMC0)
                u_k = up.tile([128, N], F32, name=f"u_{k}", tag="u")
                nsq = 2 if k == KCH - 1 else 1
                wsq = N // nsq
                for hh in range(nsq):
                    sq_i = nc.scalar.activation(
                        u_k[:, hh * wsq : (hh + 1) * wsq],
                        x_k[:, hh * wsq : (hh + 1) * wsq],
                        mybir.ActivationFunctionType.Square,
                        bias=bA_k[:],
                        scale=SQRT_N,
                    )
                    if k == 0 and hh == 0:
                        add_dep_helper(
                            sq_i.ins, pre_sq.ins, sync=False,
                            reason="table preload first",
                        )
                        add_dep_helper(
                            sq_i.ins, pre_ln.ins, sync=False,
                            reason="table preload first",
                        )
                us.append(u_k)
                Cs.append(C_k)

            psums = [
                pp.tile([OUT, 512], F32, name=f"ps_{j}", tag="ps")
                for j in range(NBLK)
            ]
            out_sb = wp.tile([OUT, N], F32, name="out_sb", tag="out_sb")
            HB = NBLK // 2  # n-blocks per half
            for h in range(2):
                for k in range(KCH):
                    l_k = lp.tile([128, N // 2], F32R, name=f"l_{h}_{k}", tag="l")
                    # finer Ln pieces on the last chunk shorten the end drain
                    nq = 2 if k == KCH - 1 else 1
                    wq = (N // 2) // nq
                    for q in range(nq):
                        nc.scalar.activation(
                            l_k[:, q * wq : (q + 1) * wq],
                            us[k][
                                :, h * (N // 2) + q * wq : h * (N // 2) + (q + 1) * wq
                            ],
                            mybir.ActivationFunctionType.Ln,
                            bias=Cs[k][:],
                            scale=EMC0,
                        )
                    for jj in range(HB):
                        j = h * HB + jj
                        nc.tensor.matmul(
                            psums[j][:],
                            lhsT=w_r[:, k * OUT : (k + 1) * OUT],
                            rhs=l_k[:, jj * 512 : (jj + 1) * 512],
                            start=(k == 0),
                            stop=(k == KCH - 1),
                        )
                for jj in range(HB):
                    j = h * HB + jj
                    if h == 0 or jj % 2 == 0:
                        nc.vector.tensor_scalar_add(
                            out_sb[:, j * 512 : (j + 1) * 512], psums[j][:], bias_b[:]
                        )
                    else:
                        nc.scalar.add(
                            out_sb[:, j * 512 : (j + 1) * 512], psums[j][:], bias_b[:]
                        )
                if h == 0:
                    nc.sync.dma_start(
                        out[:, h * (N // 2) : (h + 1) * (N // 2)],
                        out_sb[:, h * (N // 2) : (h + 1) * (N // 2)],
                    )
                else:
                    # per-bank output DMAs chase the evacuations at the tail
                    for jj in range(HB):
                        j = h * HB + jj
                        nc.sync.dma_start(
                            out[:, j * 512 : (j + 1) * 512],
                            out_sb[:, j * 512 : (j + 1) * 512],
                        )

    nc.compile()
    return nc


def _prep_inputs(data, W, b):
    data = np.ascontiguousarray(np.asarray(data, dtype=np.float32))
    W = np.asarray(W, dtype=np.float32)
    b = np.asarray(b, dtype=np.float32)
    W2T = np.ascontiguousarray(W.T * 0.5)          # [D, OUT]
    in_maps = []
    for c in range(NCORES):
        xT_c = np.ascontiguousarray(data[:, c * DC : (c + 1) * DC].T)  # [DC, N]
        wT_c = np.ascontiguousarray(W2T[c * DC : (c + 1) * DC, :])     # [DC, OUT]
        # bias per core: b/8 plus the centering correction C0*sum_d w2[d,o]
        b8_c = (b / NCORES + C0 * wT_c.sum(axis=0)).astype(np.float32)
        in_maps.append({"xT": xT_c, "wT": wT_c, "bb": np.ascontiguousarray(b8_c.reshape(OUT, 1))})
    return in_maps


def _run(inputs, trace=False, **kwargs):
    if "nc" not in _cache:
        _cache["nc"] = _build()
    nc = _cache["nc"]
    in_maps = _prep_inputs(inputs["data"], inputs["W"], inputs["b"])
    res = run_bass_kernel_spmd(
        nc, in_maps, core_ids=list(range(NCORES)), trace=trace, **kwargs
    )
    outT = np.sum([res.results[c]["out"] for c in range(NCORES)], axis=0, dtype=np.float32)
    return np.ascontiguousarray(outT.T), res


def kernel(data, W, b):
    out, _ = _run({"data": data, "W": W, "b": b})
    return out


# revision 19
# speedup vs baseline: 1.2174x; 1.0308x over previous
"""Distance-discriminator kernel for 8 Trainium2 cores.

Math (reference): for x [N, D],
    S[d] = sum_j x[j,d];  Q[d] = sum_j x[j,d]^2
    sq[i,d] = Q[d] - 2 x[i,d] S[d] + N x[i,d]^2        (= sum_j (x[j,d]-x[i,d])^2)
    out = log(sqrt(sq) + eps) @ W.T + b

Device formulation: complete the square,
    sq = (sqrt(N) x - S/sqrt(N))^2 + C,   C = Q - S^2/N
so one ACT Square pass (per-partition bias, accum_out gives N*C for free) and
one ACT Ln pass (per-partition bias C) produce logd2 = ln(sq) = 2 log(dist).
The eps and the 0.5 factor fold into the GEMM weights (W/2); eps is
negligible because dist ~ sqrt(2N) >> eps.

Sharding: columns d are split across the 8 cores (512 each), so S, Q are
fully local; each core computes a [64, 4096] partial of out.T and a
ReduceScatter sums partials, leaving rank c with out.T rows 8c:8c+8.
Inputs are transposed on the host so d sits on SBUF partitions: reductions
are free-axis, the GEMM needs no on-device transpose, and all DMA is
contiguous.
"""

import numpy as np

import concourse.bacc as bacc
import concourse.bass as bass
import concourse.tile as tile
from concourse import mybir
from concourse.tile import add_dep_helper
from concourse.bass_utils import run_bass_kernel_spmd

N = 4096          # rows
D = 4096          # feature columns
OUT = 64
NCORES = 8
DC = D // NCORES  # 512 columns per core
KCH = DC // 128   # 4 partition-chunks per core
NBLK = N // 512   # 8 moving-dim blocks per GEMM bank
SQRT_N = float(np.sqrt(N))
C0 = 8.9              # ln(sq) centering constant; absorbed via host bias
EMC0 = float(np.exp(-C0))

F32 = mybir.dt.float32
_cache: dict = {}


def _build():
    nc = bacc.Bacc(
        "TRN2",
        target_bir_lowering=False,
        debug=False,
        num_devices=NCORES,
    )
    xT = nc.dram_tensor("xT", [DC, N], F32, kind="ExternalInput").ap()
    wT = nc.dram_tensor("wT", [DC, OUT], F32, kind="ExternalInput").ap()
    bb = nc.dram_tensor("bb", [OUT, 1], F32, kind="ExternalInput").ap()
    out = nc.dram_tensor("out", [OUT, N], F32, kind="ExternalOutput").ap()

    F32R = mybir.dt.float32r
    with tile.TileContext(nc) as tc:
        with (
            tc.tile_pool(name="wp", bufs=1) as wp,
            tc.tile_pool(name="xp", bufs=KCH) as xp,
            tc.tile_pool(name="st", bufs=KCH) as st,
            tc.tile_pool(name="up", bufs=KCH) as up,
            tc.tile_pool(name="lp", bufs=3) as lp,
            tc.tile_pool(name="pp", bufs=NBLK, space="PSUM") as pp,
        ):
            # pre-load both ACT table sets (Square, Ln) while ACT is idle;
            # ordering deps below force these to schedule first
            dumm = wp.tile([128, 1], F32, name="dumm", tag="dumm")
            nc.vector.memset(dumm[:], 1.0)
            dumm2 = wp.tile([128, 1], F32, name="dumm2", tag="dumm2")
            pre_sq = nc.scalar.activation(
                dumm2[:], dumm[:], mybir.ActivationFunctionType.Square
            )
            pre_ln = nc.scalar.activation(
                dumm2[:], dumm[:], mybir.ActivationFunctionType.Ln,
                bias=dumm[:], scale=1.0,
            )

            xs = []
            di = 0
            for k in range(KCH):
                x_k = xp.tile([128, N], F32, name=f"x_{k}", tag="x")
                npieces = 8 if k == 0 else 4
                w_piece = N // npieces
                for s in range(npieces):
                    # alternate issuing engine: per-queue data BW is ~220 GB/s;
                    # scalar participates only before its ACT work starts
                    eng = nc.scalar if (di % 2 == 1 and di < 12) else nc.sync
                    di += 1
                    eng.dma_start(
                        x_k[:, s * w_piece : (s + 1) * w_piece],
                        xT[k * 128 : (k + 1) * 128, s * w_piece : (s + 1) * w_piece],
                    )
                xs.append(x_k)

            w_all = wp.tile([128, KCH * OUT], F32, name="w_all", tag="w_all")
            for k in range(KCH):
                nc.sync.dma_start(
                    w_all[:, k * OUT : (k + 1) * OUT], wT[k * 128 : (k + 1) * 128, :]
                )
            bias_b = wp.tile([OUT, 1], F32, name="bias_b", tag="bias_b")
            nc.sync.dma_start(bias_b[:], bb)
            w_r = wp.tile([128, KCH * OUT], F32R, name="w_r", tag="w_r")
            nc.vector.tensor_copy(w_r[:], w_all[:])

            us, Cs = [], []
            for k in range(KCH):
                x_k = xs[k]
                # bn_stats per 512-wide segment -> mean/var per partition
                stats_k = st.tile([128, 8, 6], F32, name=f"stats_{k}", tag="stats")
                for s in range(8):
                    nc.vector.bn_stats(
                        stats_k[:, s, :], x_k[:, s * 512 : (s + 1) * 512]
                    )
                mv_k = st.tile([128, 2], F32, name=f"mv_{k}", tag="mv")
                nc.vector.bn_aggr(mv_k[:], stats_k[:])
                # bias_A = -S/sqrt(N) = -sqrt(N)*mean ;  C = Q - S^2/N = N*var
                bA_k = st.tile([128, 1], F32, name=f"bA_{k}", tag="bA")
                nc.vector.tensor_scalar_mul(bA_k[:], mv_k[:, 0:1], -SQRT_N)
                C_k = st.tile([128, 1], F32, name=f"C_{k}", tag="C")
                nc.vector.tensor_scalar_mul(C_k[:], mv_k[:, 1:2], float(N) * EMC0)
                u_k = up.tile([128, N], F32, name=f"u_{k}", tag="u")
                if k < KCH - 1:
                    sq_i = nc.scalar.activation(
                        u_k[:],
                        x_k[:],
                        mybir.ActivationFunctionType.Square,
                        bias=bA_k[:],
                        scale=SQRT_N,
                    )
                    if k == 0:
                        add_dep_helper(
                            sq_i.ins, pre_sq.ins, sync=False,
                            reason="table preload first",
                        )
                        add_dep_helper(
                            sq_i.ins, pre_ln.ins, sync=False,
                            reason="table preload first",
                        )
                else:
                    # last chunk: h1 on ACT; h2 on DVE (idle once bn is done)
                    # to shorten the ACT critical stream
                    nc.scalar.activation(
                        u_k[:, : N // 2],
                        x_k[:, : N // 2],
                        mybir.ActivationFunctionType.Square,
                        bias=bA_k[:],
                        scale=SQRT_N,
                    )
                    v_k = lp.tile([128, N // 2], F32, name="v_k", tag="v")
                    nc.vector.tensor_scalar(
                        v_k[:], x_k[:, N // 2 :], SQRT_N, bA_k[:],
                        op0=mybir.AluOpType.mult, op1=mybir.AluOpType.add,
                    )
                    nc.vector.tensor_tensor(
                        u_k[:, N // 2 :], v_k[:], v_k[:], op=mybir.AluOpType.mult
                    )
                us.append(u_k)
                Cs.append(C_k)

            psums = [
                pp.tile([OUT, 512], F32, name=f"ps_{j}", tag="ps")
                for j in range(NBLK)
            ]
            out_sb = wp.tile([OUT, N], F32, name="out_sb", tag="out_sb")
            HB = NBLK // 2  # n-blocks per half
            for h in range(2):
                for k in range(KCH):
                    l_k = lp.tile([128, N // 2], F32R, name=f"l_{h}_{k}", tag="l")
                    # finer Ln pieces on the last chunk shorten the end drain
                    nq = 2 if k == KCH - 1 else 1
                    wq = (N // 2) // nq
                    for q in range(nq):
                        nc.scalar.activation(
                            l_k[:, q * wq : (q + 1) * wq],
                            us[k][
                                :, h * (N // 2) + q * wq : h * (N // 2) + (q + 1) * wq
                            ],
                            mybir.ActivationFunctionType.Ln,
                            bias=Cs[k][:],
                            scale=EMC0,
                        )
                    for jj in range(HB):
                        j = h * HB + jj
                        nc.tensor.matmul(
                            psums[j][:],
                            lhsT=w_r[:, k * OUT : (k + 1) * OUT],
                            rhs=l_k[:, jj * 512 : (jj + 1) * 512],
                            start=(k == 0),
                            stop=(k == KCH - 1),
                        )
                for jj in range(HB):
                    j = h * HB + jj
                    if h == 0 or jj % 2 == 0:
                        nc.vector.tensor_scalar_add(
                            out_sb[:, j * 512 : (j + 1) * 512], psums[j][:], bias_b[:]
                        )
                    else:
                        nc.scalar.add(
                            out_sb[:, j * 512 : (j + 1) * 512], psums[j][:], bias_b[:]
                        )
                if h == 0:
                    nc.sync.dma_start(
                        out[:, h * (N // 2) : (h + 1) * (N // 2)],
                        out_sb[:, h * (N // 2) : (h + 1) * (N // 2)],
                    )
                else:
                    # per-bank output DMAs chase the evacuations at the tail
                    for jj in range(HB):
                        j = h * HB + jj
                        nc.sync.dma_start(
                            out[:, j * 512 : (j + 1) * 512],
                            out_sb[:, j * 512 : (j + 1) * 512],
                        )

    nc.compile()
    return nc


def _prep_inputs(data, W, b):
    data = np.ascontiguousarray(np.asarray(data, dtype=np.float32))
    W = np.asarray(W, dtype=np.float32)
    b = np.asarray(b, dtype=np.float32)
    W2T = np.ascontiguousarray(W.T * 0.5)          # [D, OUT]
    in_maps = []
    for c in range(NCORES):
        xT_c = np.ascontiguousarray(data[:, c * DC : (c + 1) * DC].T)  # [DC, N]
        wT_c = np.ascontiguousarray(W2T[c * DC : (c + 1) * DC, :])     # [DC, OUT]
        # bias per core: b/8 plus the centering correction C0*sum_d w2[d,o]
        b8_c = (b / NCORES + C0 * wT_c.sum(axis=0)).astype(np.float32)
        in_maps.append({"xT": xT_c, "wT": wT_c, "bb": np.ascontiguousarray(b8_c.reshape(OUT, 1))})
    return in_maps


def _run(inputs, trace=False, **kwargs):
    if "nc" not in _cache:
        _cache["nc"] = _build()
    nc = _cache["nc"]
    in_maps = _prep_inputs(inputs["data"], inputs["W"], inputs["b"])
    res = run_bass_kernel_spmd(
        nc, in_maps, core_ids=list(range(NCORES)), trace=trace, **kwargs
    )
    outT = np.sum([res.results[c]["out"] for c in range(NCORES)], axis=0, dtype=np.float32)
    return np.ascontiguousarray(outT.T), res


def kernel(data, W, b):
    out, _ = _run({"data": data, "W": W, "b": b})
    return out
